# revision 33
# baseline (speedup 1.0000x reference)
"""Trainium2 Bass kernel for nn_AttentionHead (sparse causal+global attention).

Contract: kernel(**inputs) takes the FULL unsharded inputs
(q/k/v [8,2048,1024], Wq/Wk/Wv [128,1024], bq/bk/bv [128]) and returns
the FULL output [8,2048,128].

Sharding: data-parallel over batch -- one batch element per NeuronCore,
8 cores. Weights/masks replicated.

Device-side computation per core (batch element b), "transposed world":
  - host packs x[b] per sq-tile as [nj, 128, 4096] fp16 (16KB contiguous
    per-partition lines); projections (fp16 x fp16 -> f32 PSUM, bias add
    fused into the ScalarE eviction) give d-major QT/KT [128, S] fp16;
    V re-transposed on-chip (TensorE, vs a host-shipped fp16 identity)
    to s-major fp16 blocks for the AV matmul.
  - scores^T are computed in PAIRS of sk-blocks: two matmuls into one
    [128, 1024] 2-bank PSUM tile, ONE [128,1024] exp on ScalarE (the
    ACT fixed cost ~290ns/instr is the reason to pair), fp16 out.
  - causal masking is STRUCTURAL at two levels: only sk-blocks i<=4j+3
    are computed for sq-tile j, and DIAGONAL blocks skip their dead
    columns entirely (matmul N, exp, mask-mul, acc and AV all operate
    on [128*t_ : 512] slices only).
  - row sums NO LONGER burn PE streaming: each P pair is accumulated
    into a per-j [128,1024] fp16 accumulator on the DVE (one add per
    pair); a single pair of ones-matmuls per j reduces the accumulator
    over partitions. (The old per-block ones-matmul burst cost ~9.5us
    of PE streaming.)
  - AV^T[d, sq] += V_block^T @ P accumulated in PSUM over sk blocks,
    with the scores/exp stage running 2 pairs ahead of the AV consumer.
  - global tokens (32 scattered rows+cols of the SxS mask):
      B1: global KEYS (pairs sk in G, sk > sq): folded into each j's
      AV psum (VG matmul) and into the sums accumulator (DVE add);
      QG/KG/VG are host-projected (3 tiny fp32 GEMMs -> fp16 in).
      B2: global QUERIES vs non-global keys: scores+exp+mask+acc run
      interleaved per j4-group (ONE [128,128] exp per 4 blocks); only
      the 16 tiny AV matmuls + 4 sums matmuls remain in the tail.
    The active-pair sets of A/B1/B2 partition the reference mask exactly.
Host post-processing: out[b] = ((AVt [+scatter B2]) / sums).T

Scheduling/DMA notes (hard-won; several were measured the hard way):
  - ALL x input rides ONE ring (nc.sync HWDGE) in strict need order
    (k, q, v per group; q3 hoisted before v2): a single queue drives all
    16 SDMA engines at 400+ GB/s, and a late-needed v tile can never
    steal packet-round-robin bandwidth from the next group's k/q the way
    a second ring does (two-ring builds stalled the whole machine ~8us
    when k2 landed at 36us instead of 29). Constants ride the gpsimd
    SWDGE ring; outputs ride the third queue (nc.scalar) early and the
    idle sync ring late.
  - Tile dependencies follow PROGRAM ORDER: a reader emitted before its
    writer gets NO wait. All constant DMAs are emitted before the main
    loop (a bulk-weights DMA emitted after project() raced on hardware
    and produced per-core NaNs; CoreSim's uninit checker caught it).
  - projection evictions (bias add) on ScalarE: they land in projection
    phases where ACT is idle and DVE FIFO backlog would delay QT/KT
    availability (a DVE eviction for group 3 cost 13us). GpSimd tensor
    ops are 3.3x slower than DVE (1.28us vs 0.39us per [128,1024] add)
    -- only memsets ride it.
  - a fp16 identity ships FIRST in the packed constants and feeds ~30
    N=128 warm-up matmuls emitted before everything: the PE HAM clock
    gate (cold = 1.2GHz) un-throttles ~8us earlier than the baseline's
    first HAM event (20.1us). Run-to-run HAM phase is +-1.5us of noise.
  - everything is fp16 except PSUM (f32) and sums (f32 out); avt
    returns fp16 (host divides in f32). fp16 block-accumulated sums add
    ~1e-4-level err; total measured 4.5e-4, well under the 2e-2 gate.
  - PSUM budget is exactly 8 banks: score-pairs 2x2, projections 2
    (shared with the V transposes), AV 1, aux (b1 scores / per-j sums /
    B2 tails) 1.
  - measured: 93.7us baseline -> 78.7-81.8us (HAM-phase noise band);
    post-preamble span is DMA-paced to ~50us (12.6MB fp16 at ~410GB/s),
    PE/ACT/DVE all >90% busy through the attention tail, ~7us runtime
    preamble + ~10us teardown are fixed costs.
"""

import math
import os
import sys

import numpy as np

for _p in ("/opt/trn_rl_repo", "/root/.axon_site/_ro/trn_rl_repo"):
    if os.path.isdir(_p) and _p not in sys.path:
        sys.path.append(_p)

from contextlib import ExitStack

import concourse.bacc as bacc
import concourse.mybir as mybir
import concourse.tile as tile

P = 128          # partitions / head dim
C = 1024         # input channels
G = 32           # number of global tokens
SQT = 512        # sq tile width (= max fp32 moving operand / PSUM bank)
NCH = C // P     # 8 contraction chunks for projections
B = 8            # batch / cores
NWARM = 60       # HAM warm-up matmuls

F32 = mybir.dt.float32
F16 = mybir.dt.float16
AFT = mybir.ActivationFunctionType

# packed-constants column offsets (one fp16 array: identity, weights,
# ones, masks)
OFF_IDH = 0
OFF_W = {"q": P, "k": P + C, "v": P + 2 * C}
OFF_ONES = P + 3 * C
OFF_DIAG = OFF_ONES + 1
OFF_MB2 = OFF_DIAG + 4 * SQT


def _cc_cols(S):
    return OFF_MB2 + (S // P) * G


def _gtok(S):
    rng = np.random.default_rng(0)
    return rng.choice(S, size=G, replace=False)


def _host_masks(S):
    """Static 0/1 mask patterns, all tiny. float32."""
    gtok = _gtok(S)
    gset = np.zeros(S, dtype=bool)
    gset[gtok] = True
    nblk = S // P
    # 4 diagonal patterns: tile (sk_block i = 4j+t, sq_tile j):
    # active iff sq >= sk  <=>  f >= 128*t + p
    f = np.arange(SQT)[None, :]
    p = np.arange(P)[:, None]
    diag = np.stack(
        [(f >= P * t + p).astype(np.float32) for t in range(SQT // P)], axis=0
    )
    # B1: global keys, strictly above the diagonal: active iff gtok[g] > sq
    sq = np.arange(S)[None, :]
    mb1 = (gtok[:, None] > sq).astype(np.float32)  # [G, S]
    # B2: global queries vs non-global keys: active iff sk > gtok[g], sk not in G
    sk = np.arange(S)[:, None]
    mb2 = ((sk > gtok[None, :]) & ~gset[:, None]).astype(np.float32)  # [S, G]
    mb2 = np.ascontiguousarray(mb2.reshape(nblk, P, G))
    return gtok, diag, mb1, mb2


def _pack_consts(Wq, Wk, Wv, S):
    """One [128, CC_COLS] array: fp16 identity first (warm-up + V
    transposes), then per-partition-contiguous packing of the projection
    weight chunks, ones column, diag patterns and mb2."""
    _, diag, _, mb2 = _host_masks(S)
    nblk = S // P

    def wpack(W):
        wt = np.ascontiguousarray(W.T)            # [C, P] = WxT
        return np.ascontiguousarray(
            wt.reshape(NCH, P, P).transpose(1, 0, 2).reshape(P, C)
        )

    cch = np.empty((P, _cc_cols(S)), dtype=np.float16)
    cch[:, OFF_IDH : OFF_IDH + P] = np.eye(P, dtype=np.float16)
    cch[:, OFF_W["q"] : OFF_W["q"] + C] = wpack(Wq)
    cch[:, OFF_W["k"] : OFF_W["k"] + C] = wpack(Wk)
    cch[:, OFF_W["v"] : OFF_W["v"] + C] = wpack(Wv)
    cch[:, OFF_ONES] = 1.0
    cch[:, OFF_DIAG : OFF_DIAG + 4 * SQT] = diag.transpose(1, 0, 2).reshape(P, 4 * SQT)
    cch[:, OFF_MB2 : OFF_MB2 + nblk * G] = mb2.transpose(1, 0, 2).reshape(P, nblk * G)
    return cch


def build_nc(S=2048):
    """Build the single-core Bass program (SPMD across 8 cores)."""
    nblk = S // P
    nj = S // SQT
    scale = 1.0 / math.sqrt(P)

    nc = bacc.Bacc("TRN2", target_bir_lowering=False, debug=False)

    def din(name, shape, dt=F32):
        return nc.dram_tensor(name, shape, dt, kind="ExternalInput").ap()

    def dout(name, shape, dt=F32):
        return nc.dram_tensor(name, shape, dt, kind="ExternalOutput").ap()

    qt_d = din("qt", [nj, P, NCH * SQT], F16)
    kt_d = din("kt", [nj, P, NCH * SQT], F16)
    vt_d = din("vt", [nj, P, NCH * SQT], F16)
    cch_d = din("cch", [P, _cc_cols(S)], F16)
    bias_d = din("biases", [P, 3])
    mb1_d = din("mb1", [G, S], F16)
    qg_d = din("qg", [P, G], F16)   # host-projected global queries, d-major
    kg_d = din("kg", [P, G], F16)   # host-projected global keys, d-major
    vg_d = din("vg", [G, P], F16)   # host-projected global values, g-major

    avt_d = dout("avt", [P, S], F16)
    sums_d = dout("sums", [1, S])
    avb2_d = dout("avb2", [P, G])
    sumsb2_d = dout("sumsb2", [1, G])

    # ALL x input rides the single sync HWDGE ring in strict need order
    # (k, q, v per group; q3 hoisted before v2): one queue drives all 16
    # SDMA engines at full rate, and a late-needed v tile can never steal
    # bandwidth from the next group's k/q the way a second ring does.
    # Constants ride the (otherwise idle) gpsimd ring.

    with tile.TileContext(nc) as tc, ExitStack() as ctx:
        const = ctx.enter_context(tc.tile_pool(name="const", bufs=1))
        big = ctx.enter_context(tc.tile_pool(name="big", bufs=1))
        xin = ctx.enter_context(tc.tile_pool(name="xin", bufs=6))
        pp = ctx.enter_context(tc.tile_pool(name="pp", bufs=5))
        accp = ctx.enter_context(tc.tile_pool(name="accp", bufs=2))
        pb2 = ctx.enter_context(tc.tile_pool(name="pb2", bufs=4))
        ev = ctx.enter_context(tc.tile_pool(name="ev", bufs=2))
        psp = ctx.enter_context(tc.tile_pool(name="ps", bufs=2, space="PSUM"))

        # ---- constants ----
        CCh = const.tile([P, _cc_cols(S)], F16, name="CCh", tag="CCh")
        bias_sb = const.tile([P, 3], F32, name="biases", tag="biases")
        mb1_sb = const.tile([G, S], F16, name="mb1", tag="mb1")
        QG = const.tile([P, G], F16, name="QG", tag="QG")
        KG = const.tile([P, G], F16, name="KG", tag="KG")
        VG = const.tile([G, P], F16, name="VG", tag="VG")
        accb2 = const.tile([P, 4 * G], F16, name="accb2", tag="accb2")

        identh = CCh[:, OFF_IDH : OFF_IDH + P]
        ones = CCh[:, OFF_ONES : OFF_ONES + 1]
        bias = {
            "q": bias_sb[:, 0:1],
            "k": bias_sb[:, 1:2],
            "v": bias_sb[:, 2:3],
        }

        def wtile(nm, c):
            return CCh[:, OFF_W[nm] + c * P : OFF_W[nm] + (c + 1) * P]

        def diag_sl(t_, c0):
            return CCh[:, OFF_DIAG + t_ * SQT + c0 : OFF_DIAG + (t_ + 1) * SQT]

        # lead DMAs: fp16 identity + first 128 weight columns of wq/wk,
        # so warm-up matmuls and the first chunk matmuls start early.
        nc.sync.dma_start(CCh[:, 0 : P + P], cch_d[:, 0 : P + P])  # identh+wq c0
        nc.sync.dma_start(
            CCh[:, OFF_W["k"] : OFF_W["k"] + P],
            cch_d[:, OFF_W["k"] : OFF_W["k"] + P],
        )

        # ---- HAM warm-up: keep the PE busy from ~8us so the clock gate
        # opens before the DMA-paced projection matmuls arrive ----
        warm_ps = psp.tile([P, SQT], F32, name="warm", tag="av", bufs=1)
        for _ in range(NWARM):
            nc.tensor.matmul(
                warm_ps[:, 0:P], lhsT=identh, rhs=identh, start=True, stop=True
            )

        # ---- projected tensors (SBUF-resident) ----
        QT = big.tile([P, S], F16, name="QT", tag="QT")   # [d, sq]
        KT = big.tile([P, S], F16, name="KT", tag="KT")   # [d, sk]
        V = big.tile([P, S], F16, name="V", tag="V")      # 16 s-major blocks [sk,d]

        def proj_dma(nm, xd, j4):
            xt = xin.tile([P, NCH * SQT], F16, name=f"x{nm}{j4}", tag="xin")
            # piecewise load so early chunk matmuls start as pieces land
            pieces = [(0, 1), (1, 4), (4, 8)] if j4 == 0 else [(0, 4), (4, 8)]
            for a, b_ in pieces:
                nc.sync.dma_start(
                    xt[:, a * SQT : b_ * SQT], xd[j4, :, a * SQT : b_ * SQT]
                )
            return xt

        def proj_mm(nm, j4, xt, out_sb):
            psum = psp.tile([P, SQT], F32, name=f"pj{nm}{j4}", tag="pj")
            for c in range(NCH):
                nc.tensor.matmul(
                    psum[:], lhsT=wtile(nm, c), rhs=xt[:, c * SQT : (c + 1) * SQT],
                    start=(c == 0), stop=(c == NCH - 1),
                )
            # evict with per-partition bias add on ScalarE (idle in proj
            # phases; keeps DVE free for the exp->AV critical path)
            nc.scalar.activation(out_sb, psum[:], AFT.Identity, bias=bias[nm])

        def kw_aux(n):
            # keep-warm matmuls on the (early-idle) aux bank, emitted
            # BETWEEN projection groups: in the strict PE FIFO they sit
            # ahead of the next group's data-waiting chunk matmuls and
            # fill the DMA-paced holes that otherwise re-throttle HAM
            wp = psp.tile([P, P], F32, name="kw", tag="aux", bufs=1)
            for _ in range(n):
                nc.tensor.matmul(
                    wp[:], lhsT=identh, rhs=identh, start=True, stop=True
                )

        def project(nm, xd, j4, out_sb):
            proj_mm(nm, j4, proj_dma(nm, xd, j4), out_sb)
            if j4 <= 1:
                kw_aux(6)

        def v_transposes(j4, vt_tmp):
            for t_ in range(SQT // P):
                blk = j4 * (SQT // P) + t_
                pst = psp.tile([P, P], F16, name=f"vtr{blk}", tag="pj")
                nc.tensor.matmul(
                    pst[:],
                    lhsT=vt_tmp[:, t_ * P : (t_ + 1) * P],
                    rhs=identh,
                    is_transpose=True,
                )
                nc.vector.tensor_copy(V[:, blk * P : (blk + 1) * P], pst[:])

        def b1_scores(j):
            # global keys vs this sq tile (host-projected KG): one tile
            sl = slice(j * SQT, (j + 1) * SQT)
            s_ps = psp.tile([G, SQT], F32, name=f"b1s{j}", tag="aux", bufs=1)
            nc.tensor.matmul(
                s_ps[:], lhsT=KG[:], rhs=QT[:, sl], start=True, stop=True
            )
            p_sb = pp.tile([G, SQT], F16, name=f"b1p{j}", tag="b1p", bufs=2)
            nc.scalar.activation(p_sb[:], s_ps[:], AFT.Exp, scale=scale)
            nc.vector.tensor_mul(p_sb[:], p_sb[:], mb1_sb[:, sl])
            return p_sb

        accs = {}
        produced_all = {}

        def produce_unit(j, u):
            # scores pair -> one exp -> diag masks -> sums accumulate.
            # Callable OUTSIDE attention_j: units of a later tile are
            # hoisted into the previous tile's window to keep ACT fed
            # across the phase transition.
            if j not in accs:
                acc = accp.tile([P, 2 * SQT], F16, name=f"acc{j}", tag="acc")
                nc.gpsimd.memset(acc[:], 0.0)
                accs[j] = acc
                produced_all[j] = {}
            acc = accs[j]
            blocks = (2 * u, 2 * u + 1)
            c0s = [max(0, P * (i - 4 * j)) for i in blocks]
            s_ps = psp.tile([P, 2 * SQT], F32, name=f"s{j}_{u}", tag="ps")
            for h, i in enumerate(blocks):
                # the even half may skip its dead lead columns (the
                # pair ACT starts past them); the odd half streams
                # full width so the ACT never reads uninit PSUM
                cc = c0s[h] if h == 0 else 0
                nc.tensor.matmul(
                    s_ps[:, h * SQT + cc : (h + 1) * SQT],
                    lhsT=KT[:, i * P : (i + 1) * P],
                    rhs=QT[:, j * SQT + cc : (j + 1) * SQT],
                    start=True,
                    stop=True,
                )
            ppt = pp.tile([P, 2 * SQT], F16, name=f"p{j}_{u}", tag="pp")
            # ONE exp per pair; the [512 : 512+c0s[1]] slice of a
            # diagonal pair is dead (never read downstream)
            nc.scalar.activation(
                ppt[:, c0s[0] :], s_ps[:, c0s[0] :], AFT.Exp, scale=scale
            )
            for h, i in enumerate(blocks):
                t_ = i - 4 * j
                cc = c0s[h]
                if t_ >= 0:
                    psl = ppt[:, h * SQT + cc : (h + 1) * SQT]
                    nc.vector.tensor_mul(psl, psl, diag_sl(t_, cc))
            if c0s[1] == 0:
                nc.vector.tensor_add(acc[:], acc[:], ppt[:])
            else:
                for h in range(2):
                    cc = c0s[h]
                    if cc < SQT:
                        asl = acc[:, h * SQT + cc : (h + 1) * SQT]
                        nc.vector.tensor_add(
                            asl, asl, ppt[:, h * SQT + cc : (h + 1) * SQT]
                        )
            produced_all[j][u] = (ppt, c0s)

        def attention_j(j, vt_tmp, pre=0):
            # produce runs LOOK pairs ahead of the AV consumers; `pre`
            # units were already produced (hoisted into the previous
            # tile's window). B1 (global keys) is folded in as the last
            # accumulation of the AV PSUM group and a DVE add into the
            # sums accumulator.
            sl = slice(j * SQT, (j + 1) * SQT)
            nu = 2 * j + 2                       # sk-block pairs
            LOOK = 2
            av_ps = psp.tile([P, SQT], F32, name=f"av{j}", tag="av", bufs=1)
            b1p = None
            state = [pre]

            def pump():
                if state[0] < nu:
                    produce_unit(j, state[0])
                    state[0] += 1

            def consume(u):
                ppt, c0s = produced_all[j].pop(u)
                for h, i in enumerate((2 * u, 2 * u + 1)):
                    cc = c0s[h]
                    nc.tensor.matmul(
                        av_ps[:, cc:],
                        lhsT=V[:, i * P : (i + 1) * P],
                        rhs=ppt[:, h * SQT + cc : (h + 1) * SQT],
                        start=(u == 0 and h == 0),
                        stop=False,
                    )

            pump()
            pump()
            # V transposes: vt_tmp dep (ScalarE eviction) completes
            # under the first scores matmuls
            v_transposes(j, vt_tmp)
            for c in range(nu):
                consume(c)
                pump()
                if j == 3 and c == 4:
                    # b2 group 3 here: its matmuls wait on KT(3) exactly
                    # like the last produce units, so no head-of-line loss
                    b2_group(3)
                if c == max(0, nu - 3):
                    # B1 late: off the head of the produce stream
                    b1p = b1_scores(j)
                if j == 3 and c == 6:
                    b2_tail()
            nc.tensor.matmul(
                av_ps[:], lhsT=VG[:], rhs=b1p[:], start=False, stop=True
            )
            acc = accs[j]
            nc.vector.tensor_add(acc[0:G, 0:SQT], acc[0:G, 0:SQT], b1p[:])
            # sums: partition-reduce the accumulator (two matmuls into
            # one [1,512] psum group -- ~0.4us of PE instead of ~2.4)
            sm_ps = psp.tile([1, SQT], F32, name=f"sm{j}", tag="aux", bufs=1)
            nc.tensor.matmul(
                sm_ps[:], lhsT=ones, rhs=acc[:, 0:SQT], start=True, stop=False
            )
            nc.tensor.matmul(
                sm_ps[:], lhsT=ones, rhs=acc[:, SQT:], start=False, stop=True
            )
            av_sb = ev.tile([P, SQT], F16, name=f"avsb{j}", tag="ev_av")
            nc.vector.tensor_copy(av_sb[:], av_ps[:])
            if j == 3:
                # last tile: split across two queues so the final flight
                # (on the kernel-end critical path) is halved
                h0 = slice(j * SQT, j * SQT + SQT // 2)
                h1 = slice(j * SQT + SQT // 2, (j + 1) * SQT)
                nc.scalar.dma_start(avt_d[:, h0], av_sb[:, 0 : SQT // 2])
                nc.sync.dma_start(avt_d[:, h1], av_sb[:, SQT // 2 :])
            else:
                nc.scalar.dma_start(avt_d[:, sl], av_sb[:])
            sm_sb = ev.tile([1, SQT], F32, name=f"smsb{j}", tag="ev_sm")
            nc.vector.tensor_copy(sm_sb[:], sm_ps[:])
            nc.scalar.dma_start(sums_d[:, sl], sm_sb[:])

        b2ps = []

        def b2_group(g):
            # B2 scores for sk-blocks 4g..4g+3 vs the 32 global queries:
            # 4 matmuls into one psum tile, ONE [128,128] exp, one mask
            # mul, one accumulate. AV is deferred to the tail.
            s_ps = psp.tile([P, 4 * G], F32, name=f"b2s{g}", tag="ps")
            for t_ in range(4):
                i = 4 * g + t_
                nc.tensor.matmul(
                    s_ps[:, t_ * G : (t_ + 1) * G],
                    lhsT=KT[:, i * P : (i + 1) * P],
                    rhs=QG[:],
                    start=True,
                    stop=True,
                )
            p_sb = pb2.tile([P, 4 * G], F16, name=f"b2p{g}", tag="pb2")
            nc.scalar.activation(p_sb[:], s_ps[:], AFT.Exp, scale=scale)
            nc.vector.tensor_mul(
                p_sb[:],
                p_sb[:],
                CCh[:, OFF_MB2 + 4 * g * G : OFF_MB2 + (4 * g + 4) * G],
            )
            if g == 0:
                nc.vector.tensor_copy(accb2[:], p_sb[:])
            else:
                nc.vector.tensor_add(accb2[:], accb2[:], p_sb[:])
            b2ps.append(p_sb)

        def b2_tail():
            # B2 AV + sums: tiny matmuls on the aux bank, emitted inside
            # attn(3) so their output DMAs land well before kernel end
            avp = psp.tile([P, G], F32, name="b2avp", tag="aux", bufs=1)
            for i in range(nblk):
                g, t_ = divmod(i, 4)
                nc.tensor.matmul(
                    avp[:],
                    lhsT=V[:, i * P : (i + 1) * P],
                    rhs=b2ps[g][:, t_ * G : (t_ + 1) * G],
                    start=(i == 0),
                    stop=(i == nblk - 1),
                )
            av2_sb = ev.tile([P, G], F32, name="b2avsb", tag="ev_b2a")
            nc.vector.tensor_copy(av2_sb[:], avp[:])
            nc.scalar.dma_start(avb2_d[:], av2_sb[:])
            smp = psp.tile([1, G], F32, name="b2smp", tag="aux", bufs=1)
            for g in range(4):
                nc.tensor.matmul(
                    smp[:],
                    lhsT=ones,
                    rhs=accb2[:, g * G : (g + 1) * G],
                    start=(g == 0),
                    stop=(g == 3),
                )
            sm2_sb = ev.tile([1, G], F32, name="b2smsb", tag="ev_b2s")
            nc.vector.tensor_copy(sm2_sb[:], smp[:])
            nc.scalar.dma_start(sumsb2_d[:], sm2_sb[:])

        # constants ride the (otherwise idle) gpsimd ring, bulk weights
        # first. MUST be emitted before any reader (Tile deps follow
        # program order -- a reader emitted before its writer gets NO
        # wait, which was a real timing-dependent race on hardware).
        nc.gpsimd.dma_start(
            CCh[:, 2 * P : OFF_W["k"]], cch_d[:, 2 * P : OFF_W["k"]]
        )
        nc.gpsimd.dma_start(
            CCh[:, OFF_W["k"] + P : OFF_W["v"]],
            cch_d[:, OFF_W["k"] + P : OFF_W["v"]],
        )
        nc.gpsimd.dma_start(bias_sb[:], bias_d[:])
        nc.gpsimd.dma_start(
            CCh[:, OFF_W["v"] : OFF_ONES + 1],
            cch_d[:, OFF_W["v"] : OFF_ONES + 1],
        )
        nc.gpsimd.dma_start(mb1_sb[:], mb1_d[:])
        nc.gpsimd.dma_start(QG[:], qg_d[:])
        nc.gpsimd.dma_start(KG[:], kg_d[:])
        nc.gpsimd.dma_start(VG[:], vg_d[:])
        nc.gpsimd.dma_start(CCh[:, OFF_DIAG:], cch_d[:, OFF_DIAG:])

        # ---- main loop (k, q, v per group; q3's DMA hoisted before v2
        # so the ACT-heavy attn(3) phase can start as soon as k3 lands) ----
        xts = {}
        for j4 in range(nj):
            sl4 = slice(j4 * SQT, (j4 + 1) * SQT)
            if j4 == 3:
                proj_mm("q", 3, xts["q3"], QT[:, sl4])
                project("k", kt_d, 3, KT[:, sl4])
            else:
                project("k", kt_d, j4, KT[:, sl4])
                project("q", qt_d, j4, QT[:, sl4])
            if j4 == 2:
                # q3's DMA a full group early (before v2 in the ring)
                xts["q3"] = proj_dma("q", qt_d, 3)
            vt_tmp = ev.tile([P, SQT], F16, name=f"vt{j4}", tag="ev_vt")
            xts[f"v{j4}"] = proj_dma("v", vt_d, j4)
            proj_mm("v", j4, xts[f"v{j4}"], vt_tmp[:])
            attention_j(j4, vt_tmp)
            if j4 < 3:
                b2_group(j4)


    nc.compile()
    return nc


def _pack_x(xb, S):
    # [S, C] -> [nj, P, NCH*SQT] fp16: per-partition-contiguous per sq-tile
    nj = S // SQT
    return np.ascontiguousarray(
        xb.reshape(nj, SQT, NCH, P).transpose(0, 3, 2, 1).reshape(nj, P, NCH * SQT)
    ).astype(np.float16)


def _in_maps(q, k, v, Wq, bq, Wk, bk, Wv, bv, S):
    gtok, _, mb1, _ = _host_masks(S)
    shared = {
        "cch": _pack_consts(Wq, Wk, Wv, S),
        "biases": np.ascontiguousarray(
            np.stack([bq, bk, bv], axis=1).astype(np.float32)
        ),
        "mb1": mb1.astype(np.float16),
    }
    maps = []
    for b in range(q.shape[0]):
        m = dict(shared)
        m["qt"] = _pack_x(q[b], S)
        m["kt"] = _pack_x(k[b], S)
        m["vt"] = _pack_x(v[b], S)
        # global-token projections are tiny: do them on the host in fp32
        m["qg"] = np.ascontiguousarray(
            (q[b][gtok] @ Wq.T + bq).T.astype(np.float16)
        )
        m["kg"] = np.ascontiguousarray(
            (k[b][gtok] @ Wk.T + bk).T.astype(np.float16)
        )
        m["vg"] = np.ascontiguousarray(
            (v[b][gtok] @ Wv.T + bv).astype(np.float16)
        )
        maps.append(m)
    return maps


def _assemble(results, S):
    gtok = _gtok(S)
    nb = len(results)
    out = np.empty((nb, S, P), dtype=np.float32)
    for b, r in enumerate(results):
        avt = r["avt"].astype(np.float32)
        sums = r["sums"][0].copy()
        avt[:, gtok] += r["avb2"]
        sums[gtok] += r["sumsb2"][0]
        out[b] = (avt / sums[None, :]).T
    return out


_NC_CACHE = {}


def kernel(q, k, v, Wq, bq, Wk, bk, Wv, bv):
    from concourse.bass_utils import run_bass_kernel_spmd

    q = np.asarray(q, dtype=np.float32)
    k = np.asarray(k, dtype=np.float32)
    v = np.asarray(v, dtype=np.float32)
    S = q.shape[1]
    if S not in _NC_CACHE:
        _NC_CACHE[S] = build_nc(S=S)
    nc = _NC_CACHE[S]
    maps = _in_maps(
        q, k, v,
        np.asarray(Wq, np.float32), np.asarray(bq, np.float32),
        np.asarray(Wk, np.float32), np.asarray(bk, np.float32),
        np.asarray(Wv, np.float32), np.asarray(bv, np.float32),
        S,
    )
    res = run_bass_kernel_spmd(nc, maps, core_ids=list(range(len(maps))))
    return _assemble(res.results, S)


# revision 34
# speedup vs baseline: 1.1669x; 1.1669x over previous
"""Trainium2 Bass kernel for nn_AttentionHead (sparse causal+global attention).

Contract: kernel(**inputs) takes the FULL unsharded inputs
(q/k/v [8,2048,1024], Wq/Wk/Wv [128,1024], bq/bk/bv [128]) and returns
the FULL output [8,2048,128].

Sharding: data-parallel over batch -- one batch element per NeuronCore,
8 cores. Weights/masks replicated.

Device-side computation per core (batch element b), "transposed world":
  - host packs x[b] per sq-tile as [nj, 128, 4096] fp16 (16KB contiguous
    per-partition lines); projections (fp16 x fp16 -> f32 PSUM, bias add
    fused into the ScalarE eviction) give d-major QT/KT [128, S] fp16;
    V re-transposed on-chip (TensorE, vs a host-shipped fp16 identity)
    to s-major fp16 blocks for the AV matmul.
  - scores^T are computed in PAIRS of sk-blocks: two matmuls into one
    [128, 1024] 2-bank PSUM tile, ONE [128,1024] exp on ScalarE (the
    ACT fixed cost ~290ns/instr is the reason to pair), fp16 out.
  - causal masking is STRUCTURAL at two levels: only sk-blocks i<=4j+3
    are computed for sq-tile j, and DIAGONAL blocks skip their dead
    columns entirely (matmul N, exp, mask-mul, acc and AV all operate
    on [128*t_ : 512] slices only).
  - row sums NO LONGER burn PE streaming: each P pair is accumulated
    into a per-j [128,1024] fp16 accumulator on the DVE (one add per
    pair); a single pair of ones-matmuls per j reduces the accumulator
    over partitions. (The old per-block ones-matmul burst cost ~9.5us
    of PE streaming.)
  - AV^T[d, sq] += V_block^T @ P accumulated in PSUM over sk blocks,
    with the scores/exp stage running 2 pairs ahead of the AV consumer.
  - global tokens (32 scattered rows+cols of the SxS mask):
      B1: global KEYS (pairs sk in G, sk > sq): folded into each j's
      AV psum (VG matmul) and into the sums accumulator (DVE add);
      QG/KG/VG are host-projected (3 tiny fp32 GEMMs -> fp16 in).
      B2: global QUERIES vs non-global keys: scores+exp+mask+acc run
      interleaved per j4-group (ONE [128,128] exp per 4 blocks); only
      the 16 tiny AV matmuls + 4 sums matmuls remain in the tail.
    The active-pair sets of A/B1/B2 partition the reference mask exactly.
Host post-processing: out[b] = ((AVt [+scatter B2]) / sums).T

Scheduling/DMA notes (hard-won; several were measured the hard way):
  - ALL x input rides ONE ring (nc.sync HWDGE) in strict need order
    (k, q, v per group; q3 hoisted before v2): a single queue drives all
    16 SDMA engines at 400+ GB/s, and a late-needed v tile can never
    steal packet-round-robin bandwidth from the next group's k/q the way
    a second ring does (two-ring builds stalled the whole machine ~8us
    when k2 landed at 36us instead of 29). Constants ride the gpsimd
    SWDGE ring; outputs ride the third queue (nc.scalar) early and the
    idle sync ring late.
  - Tile dependencies follow PROGRAM ORDER: a reader emitted before its
    writer gets NO wait. All constant DMAs are emitted before the main
    loop (a bulk-weights DMA emitted after project() raced on hardware
    and produced per-core NaNs; CoreSim's uninit checker caught it).
  - projection evictions (bias add) on ScalarE: they land in projection
    phases where ACT is idle and DVE FIFO backlog would delay QT/KT
    availability (a DVE eviction for group 3 cost 13us). GpSimd tensor
    ops are 3.3x slower than DVE (1.28us vs 0.39us per [128,1024] add)
    -- only memsets ride it.
  - a fp16 identity ships FIRST in the packed constants and feeds ~30
    N=128 warm-up matmuls emitted before everything: the PE HAM clock
    gate (cold = 1.2GHz) un-throttles ~8us earlier than the baseline's
    first HAM event (20.1us). Run-to-run HAM phase is +-1.5us of noise.
  - everything is fp16 except PSUM (f32) and sums (f32 out); avt
    returns fp16 (host divides in f32). fp16 block-accumulated sums add
    ~1e-4-level err; total measured 4.5e-4, well under the 2e-2 gate.
  - PSUM budget is exactly 8 banks: score-pairs 2x2, projections 2
    (shared with the V transposes), AV 1, aux (b1 scores / per-j sums /
    B2 tails) 1.
  - measured: 93.7us baseline -> 78.7-81.8us (HAM-phase noise band);
    post-preamble span is DMA-paced to ~50us (12.6MB fp16 at ~410GB/s),
    PE/ACT/DVE all >90% busy through the attention tail, ~7us runtime
    preamble + ~10us teardown are fixed costs.
"""

import math
import os
import sys

import numpy as np

for _p in ("/opt/trn_rl_repo", "/root/.axon_site/_ro/trn_rl_repo"):
    if os.path.isdir(_p) and _p not in sys.path:
        sys.path.append(_p)

from contextlib import ExitStack

import concourse.bacc as bacc
import concourse.mybir as mybir
import concourse.tile as tile

P = 128          # partitions / head dim
C = 1024         # input channels
G = 32           # number of global tokens
SQT = 512        # sq tile width (= max fp32 moving operand / PSUM bank)
NCH = C // P     # 8 contraction chunks for projections
B = 8            # batch / cores
NWARM = 60       # HAM warm-up matmuls

F32 = mybir.dt.float32
F16 = mybir.dt.float16
AFT = mybir.ActivationFunctionType

# packed-constants column offsets (one fp16 array: identity, weights,
# ones, masks)
OFF_IDH = 0
OFF_W = {"q": P, "k": P + C, "v": P + 2 * C}
OFF_ONES = P + 3 * C
OFF_DIAG = OFF_ONES + 1
OFF_MB2 = OFF_DIAG + 4 * SQT


def _cc_cols(S):
    return OFF_MB2 + (S // P) * G


def _gtok(S):
    rng = np.random.default_rng(0)
    return rng.choice(S, size=G, replace=False)


def _host_masks(S):
    """Static 0/1 mask patterns, all tiny. float32."""
    gtok = _gtok(S)
    gset = np.zeros(S, dtype=bool)
    gset[gtok] = True
    nblk = S // P
    # 4 diagonal patterns: tile (sk_block i = 4j+t, sq_tile j):
    # active iff sq >= sk  <=>  f >= 128*t + p
    f = np.arange(SQT)[None, :]
    p = np.arange(P)[:, None]
    diag = np.stack(
        [(f >= P * t + p).astype(np.float32) for t in range(SQT // P)], axis=0
    )
    # B1: global keys, strictly above the diagonal: active iff gtok[g] > sq
    sq = np.arange(S)[None, :]
    mb1 = (gtok[:, None] > sq).astype(np.float32)  # [G, S]
    # B2: global queries vs non-global keys: active iff sk > gtok[g], sk not in G
    sk = np.arange(S)[:, None]
    mb2 = ((sk > gtok[None, :]) & ~gset[:, None]).astype(np.float32)  # [S, G]
    mb2 = np.ascontiguousarray(mb2.reshape(nblk, P, G))
    return gtok, diag, mb1, mb2


def _pack_consts(Wq, Wk, Wv, S):
    """One [128, CC_COLS] array: fp16 identity first (warm-up + V
    transposes), then per-partition-contiguous packing of the projection
    weight chunks, ones column, diag patterns and mb2."""
    _, diag, _, mb2 = _host_masks(S)
    nblk = S // P

    def wpack(W):
        wt = np.ascontiguousarray(W.T)            # [C, P] = WxT
        return np.ascontiguousarray(
            wt.reshape(NCH, P, P).transpose(1, 0, 2).reshape(P, C)
        )

    cch = np.empty((P, _cc_cols(S)), dtype=np.float16)
    cch[:, OFF_IDH : OFF_IDH + P] = np.eye(P, dtype=np.float16)
    cch[:, OFF_W["q"] : OFF_W["q"] + C] = wpack(Wq)
    cch[:, OFF_W["k"] : OFF_W["k"] + C] = wpack(Wk)
    cch[:, OFF_W["v"] : OFF_W["v"] + C] = wpack(Wv)
    cch[:, OFF_ONES] = 1.0
    cch[:, OFF_DIAG : OFF_DIAG + 4 * SQT] = diag.transpose(1, 0, 2).reshape(P, 4 * SQT)
    cch[:, OFF_MB2 : OFF_MB2 + nblk * G] = mb2.transpose(1, 0, 2).reshape(P, nblk * G)
    return cch


def build_nc(S=2048):
    """Build the single-core Bass program (SPMD across 8 cores)."""
    nblk = S // P
    nj = S // SQT
    scale = 1.0 / math.sqrt(P)

    nc = bacc.Bacc("TRN2", target_bir_lowering=False, debug=False)

    def din(name, shape, dt=F32):
        return nc.dram_tensor(name, shape, dt, kind="ExternalInput").ap()

    def dout(name, shape, dt=F32):
        return nc.dram_tensor(name, shape, dt, kind="ExternalOutput").ap()

    qt_d = din("qt", [nj, P, NCH * SQT], F16)
    kt_d = din("kt", [nj, P, NCH * SQT], F16)
    vt_d = din("vt", [nj, P, NCH * SQT], F16)
    cch_d = din("cch", [P, _cc_cols(S)], F16)
    bias_d = din("biases", [P, 3])
    mb1_d = din("mb1", [G, S], F16)
    qg_d = din("qg", [P, G], F16)   # host-projected global queries, d-major
    kg_d = din("kg", [P, G], F16)   # host-projected global keys, d-major
    vg_d = din("vg", [G, P], F16)   # host-projected global values, g-major

    avt_d = dout("avt", [P, S], F16)
    sums_d = dout("sums", [1, S])
    avb2_d = dout("avb2", [P, G])
    sumsb2_d = dout("sumsb2", [1, G])

    # ALL x input rides the single sync HWDGE ring in strict need order
    # (k, q, v per group; q3 hoisted before v2): one queue drives all 16
    # SDMA engines at full rate, and a late-needed v tile can never steal
    # bandwidth from the next group's k/q the way a second ring does.
    # Constants ride the (otherwise idle) gpsimd ring.

    with tile.TileContext(nc) as tc, ExitStack() as ctx:
        const = ctx.enter_context(tc.tile_pool(name="const", bufs=1))
        big = ctx.enter_context(tc.tile_pool(name="big", bufs=1))
        xin = ctx.enter_context(tc.tile_pool(name="xin", bufs=6))
        pp = ctx.enter_context(tc.tile_pool(name="pp", bufs=4))
        accp = ctx.enter_context(tc.tile_pool(name="accp", bufs=2))
        pb2 = ctx.enter_context(tc.tile_pool(name="pb2", bufs=4))
        ev = ctx.enter_context(tc.tile_pool(name="ev", bufs=2))
        psp = ctx.enter_context(tc.tile_pool(name="ps", bufs=2, space="PSUM"))

        # ---- constants ----
        CCh = const.tile([P, _cc_cols(S)], F16, name="CCh", tag="CCh")
        bias_sb = const.tile([P, 3], F32, name="biases", tag="biases")
        mb1_sb = const.tile([G, S], F16, name="mb1", tag="mb1")
        QG = const.tile([P, G], F16, name="QG", tag="QG")
        KG = const.tile([P, G], F16, name="KG", tag="KG")
        VG = const.tile([G, P], F16, name="VG", tag="VG")
        accb2 = const.tile([P, 4 * G], F16, name="accb2", tag="accb2")

        identh = CCh[:, OFF_IDH : OFF_IDH + P]
        ones = CCh[:, OFF_ONES : OFF_ONES + 1]
        bias = {
            "q": bias_sb[:, 0:1],
            "k": bias_sb[:, 1:2],
            "v": bias_sb[:, 2:3],
        }

        def wtile(nm, c):
            return CCh[:, OFF_W[nm] + c * P : OFF_W[nm] + (c + 1) * P]

        def diag_sl(t_, c0):
            return CCh[:, OFF_DIAG + t_ * SQT + c0 : OFF_DIAG + (t_ + 1) * SQT]

        # lead DMAs: fp16 identity + first 128 weight columns of wq/wk,
        # so warm-up matmuls and the first chunk matmuls start early.
        nc.sync.dma_start(CCh[:, 0 : P + P], cch_d[:, 0 : P + P])  # identh+wq c0
        nc.sync.dma_start(
            CCh[:, OFF_W["k"] : OFF_W["k"] + P],
            cch_d[:, OFF_W["k"] : OFF_W["k"] + P],
        )

        # ---- HAM warm-up: keep the PE busy from ~8us so the clock gate
        # opens before the DMA-paced projection matmuls arrive ----
        warm_ps = psp.tile([P, SQT], F32, name="warm", tag="av", bufs=1)
        for _ in range(NWARM):
            nc.tensor.matmul(
                warm_ps[:, 0:P], lhsT=identh, rhs=identh, start=True, stop=True
            )

        # ---- projected tensors (SBUF-resident) ----
        QT = big.tile([P, S], F16, name="QT", tag="QT")   # [d, sq]
        KT = big.tile([P, S], F16, name="KT", tag="KT")   # [d, sk]
        V = big.tile([P, S], F16, name="V", tag="V")      # 16 s-major blocks [sk,d]

        def proj_dma(nm, xd, j4):
            xt = xin.tile([P, NCH * SQT], F16, name=f"x{nm}{j4}", tag="xin")
            # piecewise load so early chunk matmuls start as pieces land
            pieces = [(0, 1), (1, 4), (4, 8)] if j4 == 0 else [(0, 4), (4, 8)]
            for a, b_ in pieces:
                nc.sync.dma_start(
                    xt[:, a * SQT : b_ * SQT], xd[j4, :, a * SQT : b_ * SQT]
                )
            return xt

        def proj_mm(nm, j4, xt, out_sb):
            psum = psp.tile([P, SQT], F32, name=f"pj{nm}{j4}", tag="pj")
            for c in range(NCH):
                nc.tensor.matmul(
                    psum[:], lhsT=wtile(nm, c), rhs=xt[:, c * SQT : (c + 1) * SQT],
                    start=(c == 0), stop=(c == NCH - 1),
                )
            # evict with per-partition bias add on ScalarE (idle in proj
            # phases; keeps DVE free for the exp->AV critical path)
            nc.scalar.activation(out_sb, psum[:], AFT.Identity, bias=bias[nm])

        def project(nm, xd, j4, out_sb):
            proj_mm(nm, j4, proj_dma(nm, xd, j4), out_sb)

        def v_transposes(j4, vt_tmp):
            for t_ in range(SQT // P):
                blk = j4 * (SQT // P) + t_
                pst = psp.tile([P, P], F16, name=f"vtr{blk}", tag="pj")
                nc.tensor.matmul(
                    pst[:],
                    lhsT=vt_tmp[:, t_ * P : (t_ + 1) * P],
                    rhs=identh,
                    is_transpose=True,
                )
                nc.vector.tensor_copy(V[:, blk * P : (blk + 1) * P], pst[:])

        def b1_scores(j):
            # global keys vs this sq tile (host-projected KG): one tile
            sl = slice(j * SQT, (j + 1) * SQT)
            s_ps = psp.tile([G, SQT], F32, name=f"b1s{j}", tag="aux", bufs=1)
            nc.tensor.matmul(
                s_ps[:], lhsT=KG[:], rhs=QT[:, sl], start=True, stop=True
            )
            p_sb = pp.tile([G, SQT], F16, name=f"b1p{j}", tag="b1p", bufs=2)
            nc.scalar.activation(p_sb[:], s_ps[:], AFT.Exp, scale=scale)
            nc.vector.tensor_mul(p_sb[:], p_sb[:], mb1_sb[:, sl])
            return p_sb

        accs = {}
        produced_all = {}

        def produce_unit(j, u):
            # scores pair -> one exp -> diag masks -> sums accumulate.
            # Callable OUTSIDE attention_j: units of a later tile are
            # hoisted into the previous tile's window to keep ACT fed
            # across the phase transition.
            if j not in accs:
                acc = accp.tile([P, 2 * SQT], F16, name=f"acc{j}", tag="acc")
                nc.gpsimd.memset(acc[:], 0.0)
                accs[j] = acc
                produced_all[j] = {}
            acc = accs[j]
            blocks = (2 * u, 2 * u + 1)
            c0s = [max(0, P * (i - 4 * j)) for i in blocks]
            s_ps = psp.tile([P, 2 * SQT], F32, name=f"s{j}_{u}", tag="ps")
            for h, i in enumerate(blocks):
                # the even half may skip its dead lead columns (the
                # pair ACT starts past them); the odd half streams
                # full width so the ACT never reads uninit PSUM
                cc = c0s[h] if h == 0 else 0
                nc.tensor.matmul(
                    s_ps[:, h * SQT + cc : (h + 1) * SQT],
                    lhsT=KT[:, i * P : (i + 1) * P],
                    rhs=QT[:, j * SQT + cc : (j + 1) * SQT],
                    start=True,
                    stop=True,
                )
            ppt = pp.tile([P, 2 * SQT], F16, name=f"p{j}_{u}", tag="pp")
            # ONE exp per pair; the [512 : 512+c0s[1]] slice of a
            # diagonal pair is dead (never read downstream)
            nc.scalar.activation(
                ppt[:, c0s[0] :], s_ps[:, c0s[0] :], AFT.Exp, scale=scale
            )
            for h, i in enumerate(blocks):
                t_ = i - 4 * j
                cc = c0s[h]
                if t_ >= 0:
                    psl = ppt[:, h * SQT + cc : (h + 1) * SQT]
                    nc.vector.tensor_mul(psl, psl, diag_sl(t_, cc))
            if c0s[1] == 0:
                nc.vector.tensor_add(acc[:], acc[:], ppt[:])
            else:
                for h in range(2):
                    cc = c0s[h]
                    if cc < SQT:
                        asl = acc[:, h * SQT + cc : (h + 1) * SQT]
                        nc.vector.tensor_add(
                            asl, asl, ppt[:, h * SQT + cc : (h + 1) * SQT]
                        )
            produced_all[j][u] = (ppt, c0s)

        def attention_j(j, vt_tmp, pre=0):
            # produce runs LOOK pairs ahead of the AV consumers; `pre`
            # units were already produced (hoisted into the previous
            # tile's window). B1 (global keys) is folded in as the last
            # accumulation of the AV PSUM group and a DVE add into the
            # sums accumulator.
            sl = slice(j * SQT, (j + 1) * SQT)
            nu = 2 * j + 2                       # sk-block pairs
            LOOK = 2
            av_ps = psp.tile([P, SQT], F32, name=f"av{j}", tag="av", bufs=1)
            b1p = None
            state = [pre]

            def pump():
                if state[0] < nu:
                    produce_unit(j, state[0])
                    state[0] += 1

            def consume(u):
                ppt, c0s = produced_all[j].pop(u)
                for h, i in enumerate((2 * u, 2 * u + 1)):
                    cc = c0s[h]
                    nc.tensor.matmul(
                        av_ps[:, cc:],
                        lhsT=V[:, i * P : (i + 1) * P],
                        rhs=ppt[:, h * SQT + cc : (h + 1) * SQT],
                        start=(u == 0 and h == 0),
                        stop=False,
                    )

            pump()
            pump()
            # V transposes: vt_tmp dep (ScalarE eviction) completes
            # under the first scores matmuls
            v_transposes(j, vt_tmp)
            for c in range(nu):
                consume(c)
                pump()
                if j == 3 and c == 4:
                    # b2 group 3 here: its matmuls wait on KT(3) exactly
                    # like the last produce units, so no head-of-line loss
                    b2_group(3)
                if c == max(0, nu - 3):
                    # B1 late: off the head of the produce stream
                    b1p = b1_scores(j)
                if j == 3 and c == 6:
                    b2_tail()
            nc.tensor.matmul(
                av_ps[:], lhsT=VG[:], rhs=b1p[:], start=False, stop=True
            )
            acc = accs[j]
            nc.vector.tensor_add(acc[0:G, 0:SQT], acc[0:G, 0:SQT], b1p[:])
            # sums: partition-reduce the accumulator (two matmuls into
            # one [1,512] psum group -- ~0.4us of PE instead of ~2.4)
            sm_ps = psp.tile([1, SQT], F32, name=f"sm{j}", tag="aux", bufs=1)
            nc.tensor.matmul(
                sm_ps[:], lhsT=ones, rhs=acc[:, 0:SQT], start=True, stop=False
            )
            nc.tensor.matmul(
                sm_ps[:], lhsT=ones, rhs=acc[:, SQT:], start=False, stop=True
            )
            av_sb = ev.tile([P, SQT], F16, name=f"avsb{j}", tag="ev_av")
            nc.vector.tensor_copy(av_sb[:], av_ps[:])
            if j == 3:
                # last tile: split across two queues so the final flight
                # (on the kernel-end critical path) is halved
                h0 = slice(j * SQT, j * SQT + SQT // 2)
                h1 = slice(j * SQT + SQT // 2, (j + 1) * SQT)
                nc.scalar.dma_start(avt_d[:, h0], av_sb[:, 0 : SQT // 2])
                nc.sync.dma_start(avt_d[:, h1], av_sb[:, SQT // 2 :])
            else:
                nc.scalar.dma_start(avt_d[:, sl], av_sb[:])
            sm_sb = ev.tile([1, SQT], F32, name=f"smsb{j}", tag="ev_sm")
            nc.vector.tensor_copy(sm_sb[:], sm_ps[:])
            nc.scalar.dma_start(sums_d[:, sl], sm_sb[:])

        b2ps = []

        def b2_group(g):
            # B2 scores for sk-blocks 4g..4g+3 vs the 32 global queries:
            # 4 matmuls into one psum tile, ONE [128,128] exp, one mask
            # mul, one accumulate. AV is deferred to the tail.
            s_ps = psp.tile([P, 4 * G], F32, name=f"b2s{g}", tag="ps")
            for t_ in range(4):
                i = 4 * g + t_
                nc.tensor.matmul(
                    s_ps[:, t_ * G : (t_ + 1) * G],
                    lhsT=KT[:, i * P : (i + 1) * P],
                    rhs=QG[:],
                    start=True,
                    stop=True,
                )
            p_sb = pb2.tile([P, 4 * G], F16, name=f"b2p{g}", tag="pb2")
            nc.scalar.activation(p_sb[:], s_ps[:], AFT.Exp, scale=scale)
            nc.vector.tensor_mul(
                p_sb[:],
                p_sb[:],
                CCh[:, OFF_MB2 + 4 * g * G : OFF_MB2 + (4 * g + 4) * G],
            )
            if g == 0:
                nc.vector.tensor_copy(accb2[:], p_sb[:])
            else:
                nc.vector.tensor_add(accb2[:], accb2[:], p_sb[:])
            b2ps.append(p_sb)

        def b2_tail():
            # B2 AV + sums: tiny matmuls on the aux bank, emitted inside
            # attn(3) so their output DMAs land well before kernel end
            avp = psp.tile([P, G], F32, name="b2avp", tag="aux", bufs=1)
            for i in range(nblk):
                g, t_ = divmod(i, 4)
                nc.tensor.matmul(
                    avp[:],
                    lhsT=V[:, i * P : (i + 1) * P],
                    rhs=b2ps[g][:, t_ * G : (t_ + 1) * G],
                    start=(i == 0),
                    stop=(i == nblk - 1),
                )
            av2_sb = ev.tile([P, G], F32, name="b2avsb", tag="ev_b2a")
            nc.vector.tensor_copy(av2_sb[:], avp[:])
            nc.scalar.dma_start(avb2_d[:], av2_sb[:])
            smp = psp.tile([1, G], F32, name="b2smp", tag="aux", bufs=1)
            for g in range(4):
                nc.tensor.matmul(
                    smp[:],
                    lhsT=ones,
                    rhs=accb2[:, g * G : (g + 1) * G],
                    start=(g == 0),
                    stop=(g == 3),
                )
            sm2_sb = ev.tile([1, G], F32, name="b2smsb", tag="ev_b2s")
            nc.vector.tensor_copy(sm2_sb[:], smp[:])
            nc.scalar.dma_start(sumsb2_d[:], sm2_sb[:])

        # constants ride the (otherwise idle) gpsimd ring, bulk weights
        # first. MUST be emitted before any reader (Tile deps follow
        # program order -- a reader emitted before its writer gets NO
        # wait, which was a real timing-dependent race on hardware).
        nc.gpsimd.dma_start(
            CCh[:, 2 * P : OFF_W["k"]], cch_d[:, 2 * P : OFF_W["k"]]
        )
        nc.gpsimd.dma_start(
            CCh[:, OFF_W["k"] + P : OFF_W["v"]],
            cch_d[:, OFF_W["k"] + P : OFF_W["v"]],
        )
        nc.gpsimd.dma_start(bias_sb[:], bias_d[:])
        nc.gpsimd.dma_start(
            CCh[:, OFF_W["v"] : OFF_ONES + 1],
            cch_d[:, OFF_W["v"] : OFF_ONES + 1],
        )
        nc.gpsimd.dma_start(mb1_sb[:], mb1_d[:])
        nc.gpsimd.dma_start(QG[:], qg_d[:])
        nc.gpsimd.dma_start(KG[:], kg_d[:])
        nc.gpsimd.dma_start(VG[:], vg_d[:])
        nc.gpsimd.dma_start(CCh[:, OFF_DIAG:], cch_d[:, OFF_DIAG:])

        # ---- main loop (k, q, v per group; q3's DMA hoisted before v2
        # so the ACT-heavy attn(3) phase can start as soon as k3 lands) ----
        xts = {}
        for j4 in range(nj):
            sl4 = slice(j4 * SQT, (j4 + 1) * SQT)
            if j4 == 3:
                proj_mm("q", 3, xts["q3"], QT[:, sl4])
                project("k", kt_d, 3, KT[:, sl4])
            else:
                project("k", kt_d, j4, KT[:, sl4])
                project("q", qt_d, j4, QT[:, sl4])
            if j4 == 2:
                # q3's DMA a full group early (before v2 in the ring)
                xts["q3"] = proj_dma("q", qt_d, 3)
            vt_tmp = ev.tile([P, SQT], F16, name=f"vt{j4}", tag="ev_vt")
            xts[f"v{j4}"] = proj_dma("v", vt_d, j4)
            proj_mm("v", j4, xts[f"v{j4}"], vt_tmp[:])
            attention_j(j4, vt_tmp)
            if j4 < 3:
                b2_group(j4)


    nc.compile()
    return nc


def _pack_x(xb, S):
    # [S, C] -> [nj, P, NCH*SQT] fp16: per-partition-contiguous per sq-tile
    nj = S // SQT
    return np.ascontiguousarray(
        xb.reshape(nj, SQT, NCH, P).transpose(0, 3, 2, 1).reshape(nj, P, NCH * SQT)
    ).astype(np.float16)


def _in_maps(q, k, v, Wq, bq, Wk, bk, Wv, bv, S):
    gtok, _, mb1, _ = _host_masks(S)
    shared = {
        "cch": _pack_consts(Wq, Wk, Wv, S),
        "biases": np.ascontiguousarray(
            np.stack([bq, bk, bv], axis=1).astype(np.float32)
        ),
        "mb1": mb1.astype(np.float16),
    }
    maps = []
    for b in range(q.shape[0]):
        m = dict(shared)
        m["qt"] = _pack_x(q[b], S)
        m["kt"] = _pack_x(k[b], S)
        m["vt"] = _pack_x(v[b], S)
        # global-token projections are tiny: do them on the host in fp32
        m["qg"] = np.ascontiguousarray(
            (q[b][gtok] @ Wq.T + bq).T.astype(np.float16)
        )
        m["kg"] = np.ascontiguousarray(
            (k[b][gtok] @ Wk.T + bk).T.astype(np.float16)
        )
        m["vg"] = np.ascontiguousarray(
            (v[b][gtok] @ Wv.T + bv).astype(np.float16)
        )
        maps.append(m)
    return maps


def _assemble(results, S):
    gtok = _gtok(S)
    nb = len(results)
    out = np.empty((nb, S, P), dtype=np.float32)
    for b, r in enumerate(results):
        avt = r["avt"].astype(np.float32)
        sums = r["sums"][0].copy()
        avt[:, gtok] += r["avb2"]
        sums[gtok] += r["sumsb2"][0]
        out[b] = (avt / sums[None, :]).T
    return out


_NC_CACHE = {}


def kernel(q, k, v, Wq, bq, Wk, bk, Wv, bv):
    from concourse.bass_utils import run_bass_kernel_spmd

    q = np.asarray(q, dtype=np.float32)
    k = np.asarray(k, dtype=np.float32)
    v = np.asarray(v, dtype=np.float32)
    S = q.shape[1]
    if S not in _NC_CACHE:
        _NC_CACHE[S] = build_nc(S=S)
    nc = _NC_CACHE[S]
    maps = _in_maps(
        q, k, v,
        np.asarray(Wq, np.float32), np.asarray(bq, np.float32),
        np.asarray(Wk, np.float32), np.asarray(bk, np.float32),
        np.asarray(Wv, np.float32), np.asarray(bv, np.float32),
        S,
    )
    res = run_bass_kernel_spmd(nc, maps, core_ids=list(range(len(maps))))
    return _assemble(res.results, S)
